# revision 1
# baseline (speedup 1.0000x reference)
"""Causal self-attention TRN2 kernel.

Full inputs in, full output out. Sharding: core c = 4*b + g runs batch b
(of 2) and head-group g (4 of 16 heads). Host pre-transposes each shard so
every SBUF layout is natural for the PE:

  xT  [1024, 2048] = x[b].T
  wqT/wkT/wvT [1024, 256] = w[rows of group].T   (wq pre-scaled by 1/8)
  woT [256, 1024] = wo[:, cols of group].T

Per core (all matmuls in fp32r):
  qT,kT [256,2048] = (wT).T-chunks @ xT      (contraction over D)
  v     [2048,256] = xT-chunks.T @ wvT       (natural layout, k on partition)
  ST[k,q] tiles    = kT-chunk.T @ qT-chunk   (K=64; 2 heads packed via PE
                                              row-tiles at partitions 0/64)
  E = exp(ST) on ScalarE straight from PSUM (softmax max-subtraction is
      skipped: scores are ~N(0,1), max < ~7, exp never overflows fp32);
      causal mask only on the diagonal 128x128 block of each boundary tile
      (0/1 multiply on DVE)
  AV: out.T[65,q] += [v_h | ones].T @ E      (ones column makes row 64 the
                                              softmax denominator for free)
  normalize: outer-product broadcast of 1/rowsum (K=1 matmul) and one
      tensor_tensor multiply at PSUM eviction
  y[t,:] partial = attnoutT-chunks.T @ woT   (host sums the 4 group partials)

Projection window tj and attention window qj=tj are interleaved in emission
order so ScalarE exp (the attention bottleneck) overlaps PE projection work.
"""

from contextlib import ExitStack

import numpy as np

from concourse import bacc, bass, mybir, tile
from concourse.bass_utils import run_bass_kernel_spmd
from concourse.masks import make_upper_triangular

B, T, D = 2, 2048, 1024
H, DH = 16, 64
N_CORES = 8
HG = 4                # tensor-parallel groups
HPG = H // HG         # heads per group = 4
CL = HPG * DH         # local channels = 256
KC = D // 128         # contraction chunks over D = 8
TQ = T // 512         # 512-wide T windows = 4
F32 = mybir.dt.float32
F32R = mybir.dt.float32r
PAIRED = True
ET_BUFS = 8


def r(ap):
    return ap.bitcast(F32R)


class Ctx:
    pass


def emit_consts(ctx, tc, g, wqT, wkT, wvT, woT):
    nc = tc.nc
    persist = ctx.enter_context(tc.tile_pool(name="persist", bufs=1))
    g.xt_pool = ctx.enter_context(tc.tile_pool(name="xt", bufs=3))
    g.et_pool = ctx.enter_context(tc.tile_pool(name="et", bufs=ET_BUFS))
    g.ysb_pool = ctx.enter_context(tc.tile_pool(name="ysb", bufs=4))
    g.rc_pool = ctx.enter_context(tc.tile_pool(name="rc", bufs=3))
    # One PSUM pool, 8 banks: tag "ps512" 4 slots (qk/st/y), "psB" 4 (v/av/rb)
    g.pp = ctx.enter_context(tc.tile_pool(name="pp", bufs=4, space="PSUM"))

    g.mask01 = persist.tile([128, 128], F32, tag="mask01", name="mask01")
    make_upper_triangular(nc, g.mask01[:, :], val=1.0, diag=True)

    # memset cannot write f32r: stage ones in f32 and copy (copy = rounding
    # producer for the fp32r matmul inputs)
    ones_f32 = persist.tile([128, 4], F32, tag="ones_f32", name="ones_f32")
    nc.vector.memset(ones_f32[:, :], 1.0)
    g.ones_col = persist.tile([1, 64], F32R, tag="ones", name="ones")
    nc.vector.tensor_copy(g.ones_col[:, :], ones_f32[0:1, 0:1].broadcast_to([1, 64]))

    # merged weight tiles: chunk kc of wX lives at cols CL*kc (one DMA each)
    g.wq_all = persist.tile([128, KC * CL], F32R, tag="wq_all", name="wq_all")
    g.wk_all = persist.tile([128, KC * CL], F32R, tag="wk_all", name="wk_all")
    g.wv_all = persist.tile([128, KC * CL], F32R, tag="wv_all", name="wv_all")
    g.wo_all = persist.tile([128, 2 * D], F32R, tag="wo_all", name="wo_all")
    g.wq_sb = [g.wq_all[:, CL * i:CL * i + CL] for i in range(KC)]
    g.wk_sb = [g.wk_all[:, CL * i:CL * i + CL] for i in range(KC)]
    g.wv_sb = [g.wv_all[:, CL * i:CL * i + CL] for i in range(KC)]
    g.wo_sb = [g.wo_all[:, D * i:D * i + D] for i in range(2)]
    # weight DMAs are issued inside emit_proj(0) (after the first x window,
    # interleaved per projection) so the PE can start ~2us into the kernel

    g.qT_sb = [persist.tile([128, T], F32R, tag=f"qT{i}", name=f"qT{i}") for i in range(2)]
    g.kT_sb = [persist.tile([128, T], F32R, tag=f"kT{i}", name=f"kT{i}") for i in range(2)]
    g.aT_sb = [persist.tile([128, T], F32R, tag=f"aT{i}", name=f"aT{i}") for i in range(2)]

    # v natural layout, one tile per 128-row k-chunk, head-strided cols of 65
    # (col 65h+64 is the ones column for the softmax denominator trick)
    g.v_sb = [persist.tile([128, HPG * 65], F32R, tag=f"v{i}", name=f"v{i}")
              for i in range(T // 128)]
    for i in range(T // 128):
        ones_cols = g.v_sb[i].rearrange("p (h c) -> p h c", c=65)[:, :, 64:65]
        nc.vector.tensor_copy(ones_cols, ones_f32.rearrange("p (h c) -> p h c", c=1))


def emit_proj(tc, g, xT, tj, wqT=None, wkT=None):
    nc = tc.nc
    ts = 512 * tj
    xt_all = g.xt_pool.tile([128, KC * 512], F32R, tag="xt", name="xt")
    for half in range(2):  # two DMAs: finer dependency pacing, few dispatches
        nc.sync.dma_start(
            out=xt_all.rearrange("p (kc t) -> p kc t", t=512)[:, 4 * half:4 * half + 4],
            in_=xT.rearrange("(kc p) t -> p kc t", p=128)[:, 4 * half:4 * half + 4,
                                                          ts:ts + 512],
        )
    xt = [xt_all[:, 512 * kc:512 * kc + 512] for kc in range(KC)]

    for (w_sb, dst, wT, w_all) in ((g.wq_sb, g.qT_sb, wqT, g.wq_all),
                                   (g.wk_sb, g.kT_sb, wkT, g.wk_all)):
        if wT is not None:  # first window: load this projection's weights now
            nc.scalar.dma_start(
                out=w_all.rearrange("p (kc c) -> p kc c", c=CL),
                in_=wT.rearrange("(kc p) c -> p kc c", p=128),
            )
        for m in range(2):
            # window 0: the av slots are idle until the first AV matmul
            # (which waits on v-proj anyway) -- borrow them so the four
            # startup q/k PSUM groups double-buffer instead of serializing
            if tj == 0:
                ps = g.pp.tile([128, 512], F32, tag="av", bufs=2, name="psqk")
            else:
                ps = g.pp.tile([128, 512], F32, tag="pj", bufs=1, name="psqk")
            for kc in range(KC):
                nc.tensor.matmul(
                    out=ps[:, :],
                    lhsT=(w_sb[kc][:, 128 * m:128 * m + 128]),
                    rhs=(xt[kc][:, :]),
                    start=(kc == 0),
                    stop=(kc == KC - 1),
                )
            nc.vector.tensor_copy(dst[m][:, ts:ts + 512], ps[:, :])
    return xt_all


def emit_proj_v(tc, g, tj, xt_all, wvT=None, woT=None):
    nc = tc.nc
    xt = [xt_all[:, 512 * kc:512 * kc + 512] for kc in range(KC)]
    if wvT is not None:
        nc.scalar.dma_start(
            out=g.wv_all.rearrange("p (kc c) -> p kc c", c=CL),
            in_=wvT.rearrange("(kc p) c -> p kc c", p=128),
        )
    for tc4 in range(4):
        tg = 4 * tj + tc4
        ps = g.pp.tile([128, CL], F32, tag="pj", bufs=1, name="psv")
        for kc in range(KC):
            nc.tensor.matmul(
                out=ps[:, :],
                lhsT=(xt[kc][:, 128 * tc4:128 * tc4 + 128]),
                rhs=(g.wv_sb[kc][:, :]),
                start=(kc == 0),
                stop=(kc == KC - 1),
            )
        nc.vector.tensor_copy(
            g.v_sb[tg].rearrange("p (h c) -> p h c", c=65)[:, :, 0:64],
            ps.rearrange("p (h c) -> p h c", c=64)[:, :, :],
        )
    if woT is not None:  # needed only by the first output projection
        nc.scalar.dma_start(
            out=g.wo_all.rearrange("p (cc d) -> p cc d", d=D),
            in_=woT.rearrange("(cc p) d -> p cc d", p=128),
        )


def emit_attn(tc, g, y, qj, phase="all", stash=None):
    nc = tc.nc
    qs = 512 * qj
    nk = 4 * qj + 4  # k-chunks 0..nk-1 reach this window

    def geom(ki):
        if ki < 4 * qj:
            return 512, 0
        w = 512 - 128 * (ki - 4 * qj)
        return w, 512 - w

    for hp in range(2):  # head pair -> partitions 0:64 / 64:128 of tile hp
        if phase != "scores":
            av = [g.pp.tile([65, 512], F32, tag="av", bufs=2, name="av")
                  for _ in range(2)]
        npair = nk // 2 if PAIRED else nk
        for pi in range(npair):
            if PAIRED:
                ki0, ki1 = 2 * pi, 2 * pi + 1
            else:
                ki0 = ki1 = pi
            w0, qoff0 = geom(ki0)
            w1, qoff1 = geom(ki1)
            if phase == "av":
                ets = stash[(hp, pi)]
            else:
                ets = []
                for hh in range(2):  # packed PE row-tiles (base partition 0/64)
                    po = 64 * hh
                    if PAIRED:
                        st = g.pp.tile([128, 1024], F32, tag="st", bufs=2, name="st")
                        plan = ((ki0, w0, qoff0, 0), (ki1, w1, qoff1, w0))
                    else:
                        st = g.pp.tile([128, 512], F32, tag="st", bufs=4, name="st")
                        plan = ((ki0, w0, qoff0, 0),)
                    for (ki, w, qoff, co) in plan:
                        nc.tensor.matmul(
                            out=st[:, co:co + w],
                            lhsT=(g.kT_sb[hp][po:po + 64, 128 * ki:128 * ki + 128]),
                            rhs=(g.qT_sb[hp][po:po + 64, qs + qoff:qs + 512]),
                            start=True,
                            stop=True,
                        )
                    wid = w0 + w1 if PAIRED else w0
                    et = g.et_pool.tile([128, 1024], F32R, tag="et", name="et")
                    nc.scalar.activation(
                        out=et[:, :wid],
                        in_=st[:, :wid],
                        func=mybir.ActivationFunctionType.Exp,
                    )
                    if ki0 >= 4 * qj:  # diagonal 128x128 blocks need the mask
                        nc.vector.tensor_mul(et[:, 0:128], et[:, 0:128],
                                             g.mask01[:, :])
                    if PAIRED and ki1 >= 4 * qj:
                        nc.vector.tensor_mul(et[:, w0:w0 + 128], et[:, w0:w0 + 128],
                                             g.mask01[:, :])
                    ets.append(et)
                if phase == "scores":
                    stash[(hp, pi)] = ets
                    continue
            for hh in range(2):
                h = 2 * hp + hh
                nc.tensor.matmul(
                    out=av[hh][:, qoff0:512],
                    lhsT=(g.v_sb[ki0][:, 65 * h:65 * h + 65]),
                    rhs=(ets[hh][:, :w0]),
                    start=(ki0 == 0),
                    stop=(not PAIRED and ki0 == nk - 1),
                )
                if PAIRED:
                    nc.tensor.matmul(
                        out=av[hh][:, qoff1:512],
                        lhsT=(g.v_sb[ki1][:, 65 * h:65 * h + 65]),
                        rhs=(ets[hh][:, w0:w0 + w1]),
                        start=False,
                        stop=(ki1 == nk - 1),
                    )
        if phase == "scores":
            continue
        for hh in range(2):
            po = 64 * hh
            recip_r = g.rc_pool.tile([1, 512], F32R, tag="recip", name="recip")
            with nc.allow_low_precision(reason="fp32r softmax denominator"):
                nc.vector.reciprocal(recip_r[:, :], av[hh][64:65, :])
            rb = g.pp.tile([64, 512], F32, tag="pj", bufs=1, name="rb")
            nc.tensor.matmul(
                out=rb[:, :],
                lhsT=(g.ones_col[:, :]),
                rhs=(recip_r[:, :]),
                start=True,
                stop=True,
            )
            rb_sb = g.rc_pool.tile([64, 512], F32, tag="rb_sb", name="rb_sb")
            nc.vector.tensor_copy(rb_sb[:, :], rb[:, :])
            nc.vector.tensor_mul(
                g.aT_sb[hp][po:po + 64, qs:qs + 512], av[hh][0:64, :], rb_sb[:, :]
            )


def emit_outproj(tc, g, y, qj):
    nc = tc.nc
    for tc4 in range(4):
        tg = 4 * qj + tc4
        ysb = g.ysb_pool.tile([128, D], F32, tag="ysb", name="ysb")
        for dj in range(2):
            py = g.pp.tile([128, 512], F32, tag="py", bufs=1, name="py")
            for cc in range(2):
                nc.tensor.matmul(
                    out=py[:, :],
                    lhsT=(g.aT_sb[cc][:, 128 * tg:128 * tg + 128]),
                    rhs=(g.wo_sb[cc][:, 512 * dj:512 * dj + 512]),
                    start=(cc == 0),
                    stop=(cc == 1),
                )
            nc.vector.tensor_copy(ysb[:, 512 * dj:512 * dj + 512], py[:, :])
        nc.sync.dma_start(out=y[128 * tg:128 * tg + 128, :], in_=ysb[:, :])


def attn_kernel(ctx, tc, y, xT, wqT, wkT, wvT, woT, n_reps=1):
    g = Ctx()
    emit_consts(ctx, tc, g, wqT, wkT, wvT, woT)
    for rep in range(n_reps):
        for w in range(TQ):
            first = rep == 0 and w == 0
            if first:
                xt_all = emit_proj(tc, g, xT, w, wqT, wkT)
                stash = {}
                emit_attn(tc, g, y, w, phase="scores", stash=stash)
                emit_proj_v(tc, g, w, xt_all, wvT=wvT, woT=woT)
                emit_attn(tc, g, y, w, phase="av", stash=stash)
            else:
                xt_all = emit_proj(tc, g, xT, w)
                emit_proj_v(tc, g, w, xt_all)
                emit_attn(tc, g, y, w)
            emit_outproj(tc, g, y, w)
    return


_PROGRAMS = {}


def get_program(n_reps=1):
    key = (n_reps, PAIRED, ET_BUFS)
    if key not in _PROGRAMS:
        nc = bacc.Bacc("TRN2", target_bir_lowering=False, debug=False,
                       num_devices=N_CORES)
        xT = nc.dram_tensor("xT", [D, T], F32R, kind="ExternalInput").ap()
        wqT = nc.dram_tensor("wqT", [D, CL], F32R, kind="ExternalInput").ap()
        wkT = nc.dram_tensor("wkT", [D, CL], F32R, kind="ExternalInput").ap()
        wvT = nc.dram_tensor("wvT", [D, CL], F32R, kind="ExternalInput").ap()
        woT = nc.dram_tensor("woT", [CL, D], F32R, kind="ExternalInput").ap()
        y = nc.dram_tensor("y", [T, D], F32, kind="ExternalOutput").ap()
        with tile.TileContext(nc) as tc:
            with ExitStack() as ctx:
                attn_kernel(ctx, tc, y, xT, wqT, wkT, wvT, woT, n_reps=n_reps)
        nc.compile()
        _PROGRAMS[key] = nc
    return _PROGRAMS[key]


def get_trivial_program():
    """Minimal NEFF with the same I/O signature, for dispatch-overhead
    baseline measurement."""
    if "trivial" not in _PROGRAMS:
        nc = bacc.Bacc("TRN2", target_bir_lowering=False, debug=False,
                       num_devices=N_CORES)
        xT = nc.dram_tensor("xT", [D, T], F32R, kind="ExternalInput").ap()
        nc.dram_tensor("wqT", [D, CL], F32R, kind="ExternalInput")
        nc.dram_tensor("wkT", [D, CL], F32R, kind="ExternalInput")
        nc.dram_tensor("wvT", [D, CL], F32R, kind="ExternalInput")
        nc.dram_tensor("woT", [CL, D], F32R, kind="ExternalInput")
        y = nc.dram_tensor("y", [T, D], F32, kind="ExternalOutput").ap()
        with tile.TileContext(nc) as tc:
            with ExitStack() as ctx:
                pool = ctx.enter_context(tc.tile_pool(name="t", bufs=1))
                t = pool.tile([128, 512], F32R, tag="t", name="t")
                o = pool.tile([128, 512], F32, tag="o", name="o")
                nc.sync.dma_start(out=t[:, :], in_=xT[0:128, 0:512])
                nc.vector.tensor_copy(o[:, :], t[:, :])
                nc.sync.dma_start(out=y[0:128, 0:512], in_=o[:, :])
        nc.compile()
        _PROGRAMS["trivial"] = nc
    return _PROGRAMS["trivial"]


def make_in_maps(x, wq, wk, wv, wo):
    x = np.asarray(x, np.float32)
    wq, wk, wv, wo = (np.asarray(a, np.float32) for a in (wq, wk, wv, wo))
    scale = np.float32(DH ** -0.5)
    in_maps = []
    for c in range(N_CORES):
        b, g = divmod(c, HG)
        rows = slice(g * CL, (g + 1) * CL)
        in_maps.append({
            "xT": np.ascontiguousarray(x[b].T),
            # score scale 1/sqrt(DH) folded into wq on the host
            "wqT": np.ascontiguousarray(wq[rows].T) * scale,
            "wkT": np.ascontiguousarray(wk[rows].T),
            "wvT": np.ascontiguousarray(wv[rows].T),
            "woT": np.ascontiguousarray(wo[:, rows].T),
        })
    return in_maps


def gather(results):
    y = np.zeros((B, T, D), np.float32)
    for c in range(N_CORES):
        y[c // HG] += results[c]["y"]
    return y


def kernel(x, wq, wk, wv, wo):
    nc = get_program()
    in_maps = make_in_maps(x, wq, wk, wv, wo)
    res = run_bass_kernel_spmd(nc, in_maps, list(range(N_CORES)))
    return gather(res.results)



# revision 5
# speedup vs baseline: 1.0010x; 1.0010x over previous
"""Causal self-attention TRN2 kernel (bf16 rewrite).

Full inputs in, full output out. Sharding: core c = 4*b + g runs batch b
(of 2) and head-group g (4 of 16 heads). Host pre-transposes + pre-casts
each shard to bf16 so every SBUF layout is natural for the PE:

  xT  [1024, 2048] bf16 = x[b].T
  wqT/wkT/wvT [1024, 256] bf16 = w[rows of group].T  (wq pre-scaled 1/8)
  woT [256, 1024] bf16 = wo[:, cols of group].T

All matmul operands are bf16 (moving-operand dtype sets the PE rate:
1 cycle/row at ANY width, vs fp32r's 4x penalty under 256 wide), PSUM
accumulates fp32. Per core:

  qT,kT [256,2048] = (wT).T-chunks @ xT       (evicted to bf16 SBUF, DVE)
  v     [2048,260] = xT-chunks.T @ wvT        (65-col head stride; col 64
                                               is ones = softmax denom)
  ST[k,q] tiles    = kT-chunk.T @ qT-chunk    (K=64; 2 heads at PE row
                                               tiles 0/64; 2 k-chunks
                                               packed per [128,1024] PSUM)
  E = exp(ST) on ScalarE PSUM->SBUF bf16 (max-subtraction skipped:
      scores ~N(0,1), exp never overflows); causal mask = 0/1 bf16
      multiply on GPSIMD (SBUF only) for diagonal 128-blocks
  AV: out.T[65,q] += [v_h | ones].T @ E
  normalize: recip (DVE) -> partition_broadcast (GPSIMD) -> one
      tensor_mul at PSUM eviction (DVE), writing aT bf16
  y[t,:] partial = aT-chunks.T @ woT           (host sums 4 group partials)

Emission order targets a stall-free PE stream: per window, qk-proj
groups (pj PSUM, 2 bufs) interleave with v-proj blocks (borrowing the
av PSUM tag); attention pairs run with a 2-deep score->exp->av software
pipeline; the PREVIOUS window's output projection is emitted in dj-half
"filler" slots spread through the pair stream so the PE keeps working
while ScalarE exp (slower per pair than the PE) catches up.
"""

from collections import deque
from contextlib import ExitStack

import ml_dtypes
import numpy as np

from concourse import bacc, bass, mybir, tile
from concourse.bass_utils import run_bass_kernel_spmd
from concourse.masks import make_upper_triangular

B, T, D = 2, 2048, 1024
H, DH = 16, 64
N_CORES = 8
HG = 4                # tensor-parallel groups
HPG = H // HG         # heads per group = 4
CL = HPG * DH         # local channels = 256
KC = D // 128         # contraction chunks over D = 8
TQ = T // 512         # 512-wide T windows = 4
F32 = mybir.dt.float32
F32R = mybir.dt.float32r
BF16 = mybir.dt.bfloat16
ET_BUFS = 8
USE_POOL_BCAST = True


class Ctx:
    pass


def emit_consts(ctx, tc, g):
    nc = tc.nc
    persist = ctx.enter_context(tc.tile_pool(name="persist", bufs=1))
    g.xt_pool = ctx.enter_context(tc.tile_pool(name="xt", bufs=3))
    g.et_pool = ctx.enter_context(tc.tile_pool(name="et", bufs=ET_BUFS))
    g.ysb_pool = ctx.enter_context(tc.tile_pool(name="ysb", bufs=4))
    g.rc_pool = ctx.enter_context(tc.tile_pool(name="rc", bufs=4))
    # PSUM, 8 banks: st 2x[128,1024] (4), av 2x[65,512]|[128,256] (2),
    # pj 2x[128,512] (2) shared by qk-proj groups and outproj dj-halves
    g.pp = ctx.enter_context(tc.tile_pool(name="pp", bufs=4, space="PSUM"))

    g.mask01 = persist.tile([128, 128], BF16, tag="mask01", name="mask01")
    make_upper_triangular(nc, g.mask01[:, :], val=1.0, diag=True)

    ones_f32 = persist.tile([128, 4], F32, tag="ones_f32", name="ones_f32")
    nc.vector.memset(ones_f32[:, :], 1.0)

    g.wq_all = persist.tile([128, KC * CL], BF16, tag="wq_all", name="wq_all")
    g.wk_all = persist.tile([128, KC * CL], BF16, tag="wk_all", name="wk_all")
    g.wv_all = persist.tile([128, KC * CL], BF16, tag="wv_all", name="wv_all")
    g.wo_all = persist.tile([128, 2 * D], BF16, tag="wo_all", name="wo_all")
    g.wq_sb = [g.wq_all[:, CL * i:CL * i + CL] for i in range(KC)]
    g.wk_sb = [g.wk_all[:, CL * i:CL * i + CL] for i in range(KC)]
    g.wv_sb = [g.wv_all[:, CL * i:CL * i + CL] for i in range(KC)]
    g.wo_sb = [g.wo_all[:, D * i:D * i + D] for i in range(2)]

    g.qT_sb = [persist.tile([128, T], BF16, tag=f"qT{i}", name=f"qT{i}") for i in range(2)]
    g.kT_sb = [persist.tile([128, T], BF16, tag=f"kT{i}", name=f"kT{i}") for i in range(2)]
    g.aT_sb = [persist.tile([128, T], BF16, tag=f"aT{i}", name=f"aT{i}") for i in range(2)]

    # v natural layout, one tile per 128-row k-chunk, head-strided cols of 65
    # (col 65h+64 is the ones column for the softmax denominator trick)
    g.v_sb = [persist.tile([128, HPG * 65], BF16, tag=f"v{i}", name=f"v{i}")
              for i in range(T // 128)]
    for i in range(T // 128):
        ones_cols = g.v_sb[i].rearrange("p (h c) -> p h c", c=65)[:, :, 64:65]
        nc.vector.tensor_copy(ones_cols, ones_f32.rearrange("p (h c) -> p h c", c=1))

    if not USE_POOL_BCAST:
        # stationary selector for the combined-hh reciprocal broadcast:
        # row 0 -> partitions 0:64, row 1 -> partitions 64:128
        sel_f32 = persist.tile([2, 128], F32, tag="sel_f32", name="sel_f32")
        nc.vector.memset(sel_f32[:, :], 0.0)
        nc.vector.memset(sel_f32[0:1, 0:64], 1.0)
        nc.vector.memset(sel_f32[1:2, 64:128], 1.0)
        g.sel2 = persist.tile([2, 128], BF16, tag="sel2", name="sel2")
        nc.vector.tensor_copy(g.sel2[:, :], sel_f32[:, :])

    g.fillers = deque()


def emit_proj(tc, g, xT, w, wqT=None, wkT=None, wvT=None, woT=None):
    """qk-proj groups interleaved with v-proj blocks; first window also
    issues the weight DMAs (halved so the first matmuls start early)."""
    nc = tc.nc
    ts = 512 * w
    xt_all = g.xt_pool.tile([128, KC * 512], BF16, tag="xt", name="xt")
    xr = xt_all.rearrange("p (kc t) -> p kc t", t=512)
    xs = xT.rearrange("(kc p) t -> p kc t", p=128)
    for hq in range(4):
        nc.sync.dma_start(out=xr[:, 2 * hq:2 * hq + 2],
                          in_=xs[:, 2 * hq:2 * hq + 2, ts:ts + 512])
    xt = [xt_all[:, 512 * kc:512 * kc + 512] for kc in range(KC)]

    if wqT is not None:
        for (w_all, wT) in ((g.wq_all, wqT), (g.wk_all, wkT), (g.wv_all, wvT)):
            wr = w_all.rearrange("p (kc c) -> p kc c", c=CL)
            wsrc = wT.rearrange("(kc p) c -> p kc c", p=128)
            for h in range(2):
                nc.scalar.dma_start(out=wr[:, 4 * h:4 * h + 4],
                                    in_=wsrc[:, 4 * h:4 * h + 4])
        nc.scalar.dma_start(out=g.wo_all.rearrange("p (cc d) -> p cc d", d=D),
                            in_=woT.rearrange("(cc p) d -> p cc d", p=128))

    def qk_group(w_sb, dst, m):
        ps = g.pp.tile([128, 512], F32, tag="pj", bufs=2, name="psqk")
        for kc in range(KC):
            nc.tensor.matmul(out=ps[:, :],
                             lhsT=w_sb[kc][:, 128 * m:128 * m + 128],
                             rhs=xt[kc][:, :],
                             start=(kc == 0), stop=(kc == KC - 1))
        nc.vector.tensor_copy(dst[m][:, ts:ts + 512], ps[:, :])

    def v_block(tc4):
        tg = 4 * w + tc4
        ps = g.pp.tile([128, CL], F32, tag="av", bufs=2, name="psv")
        for kc in range(KC):
            nc.tensor.matmul(out=ps[:, :],
                             lhsT=xt[kc][:, 128 * tc4:128 * tc4 + 128],
                             rhs=g.wv_sb[kc][:, :],
                             start=(kc == 0), stop=(kc == KC - 1))
        nc.vector.tensor_copy(
            g.v_sb[tg].rearrange("p (h c) -> p h c", c=65)[:, :, 0:64],
            ps.rearrange("p (h c) -> p h c", c=64)[:, :, :],
        )

    qk_group(g.wq_sb, g.qT_sb, 0)
    v_block(0)
    qk_group(g.wk_sb, g.kT_sb, 0)
    v_block(1)
    qk_group(g.wq_sb, g.qT_sb, 1)
    v_block(2)
    qk_group(g.wk_sb, g.kT_sb, 1)
    v_block(3)


def emit_attn(tc, g, w):
    """Pair stream for window w: scores -> exp -> av with a 2-deep pipeline,
    previous-window outproj dj-halves as PE fillers, per-hp normalization."""
    nc = tc.nc
    qs = 512 * w
    nk = 4 * w + 4

    def geom(ki):
        if ki < 4 * w:
            return 512, 0
        wd = 512 - 128 * (ki - 4 * w)
        return wd, 512 - wd

    slots = [2 * (2 * w + 2)]  # av slots left in this window (both hps)

    def pop_fillers():
        # spread pending fillers evenly over the window's remaining av slots
        slots[0] -= 1
        nf = len(g.fillers)
        if not nf or slots[0] <= 0:
            return
        k = (nf + slots[0] - 1) // slots[0]
        for _ in range(min(k, nf)):
            g.fillers.popleft()()

    def emit_scores(hp, ki0, ki1):
        w0, qoff0 = geom(ki0)
        w1, _ = geom(ki1)
        wid = w0 + w1
        ets = []
        for hh in range(2):
            po = 64 * hh
            st = g.pp.tile([128, 1024], F32, tag="st", bufs=2, name="st")
            co = 0
            for ki in (ki0, ki1):
                wd, qoff = geom(ki)
                nc.tensor.matmul(
                    out=st[:, co:co + wd],
                    lhsT=g.kT_sb[hp][po:po + 64, 128 * ki:128 * ki + 128],
                    rhs=g.qT_sb[hp][po:po + 64, qs + qoff:qs + 512],
                    start=True, stop=True)
                co += wd
            et = g.et_pool.tile([128, 1024], BF16, tag="et", name="et")
            nc.scalar.activation(out=et[:, :wid], in_=st[:, :wid],
                                 func=mybir.ActivationFunctionType.Exp)
            if ki0 >= 4 * w:
                nc.gpsimd.tensor_mul(et[:, 0:128], et[:, 0:128], g.mask01[:, :])
            if ki1 >= 4 * w:
                nc.gpsimd.tensor_mul(et[:, w0:w0 + 128], et[:, w0:w0 + 128],
                                     g.mask01[:, :])
            ets.append(et)
        return (ets, ki0, ki1, w0, w1, qoff0)

    def emit_av(av, hp, item):
        (ets, ki0, ki1, w0, w1, qoff0) = item
        qoff1 = 512 - w1
        for hh in range(2):
            h = 2 * hp + hh
            nc.tensor.matmul(
                out=av[hh][:, qoff0:512],
                lhsT=g.v_sb[ki0][:, 65 * h:65 * h + 65],
                rhs=ets[hh][:, :w0],
                start=(ki0 == 0), stop=False)
            nc.tensor.matmul(
                out=av[hh][:, qoff1:512],
                lhsT=g.v_sb[ki1][:, 65 * h:65 * h + 65],
                rhs=ets[hh][:, w0:w0 + w1],
                start=False, stop=(ki1 == nk - 1))

    def emit_norm(hp, av):
        if USE_POOL_BCAST:
            for hh in range(2):
                po = 64 * hh
                rc = g.rc_pool.tile([1, 512], F32, tag="rc", name="rc")
                nc.vector.reciprocal(rc[:, :], av[hh][64:65, :])
                rbb = g.rc_pool.tile([64, 512], F32, tag="rbb", name="rbb")
                nc.gpsimd.partition_broadcast(rbb[:, :], rc[:, :], channels=64)
                nc.vector.tensor_mul(
                    g.aT_sb[hp][po:po + 64, qs:qs + 512], av[hh][0:64, :],
                    rbb[:, :])
            return
        rc2 = g.rc_pool.tile([2, 512], F32, tag="rc2", name="rc2")
        for hh in range(2):
            nc.vector.reciprocal(rc2[hh:hh + 1, :], av[hh][64:65, :])
        rb = g.pp.tile([128, 512], F32, tag="pj", bufs=2, name="rb")
        nc.tensor.matmul(out=rb[:, :], lhsT=g.sel2[:, :],
                         rhs=rc2[:, :].bitcast(F32R), start=True, stop=True)
        rb_sb = g.rc_pool.tile([128, 512], F32, tag="rb_sb", name="rb_sb")
        nc.vector.tensor_copy(rb_sb[:, :], rb[:, :])
        for hh in range(2):
            po = 64 * hh
            nc.vector.tensor_mul(
                g.aT_sb[hp][po:po + 64, qs:qs + 512], av[hh][0:64, :],
                rb_sb[po:po + 64, :])

    for hp in range(2):
        av = [g.pp.tile([65, 512], F32, tag="av", bufs=2, name="av")
              for _ in range(2)]
        pairs = [(2 * i, 2 * i + 1) for i in range(2 * w)] \
            + [(nk - 4, nk - 3), (nk - 2, nk - 1)]
        pend = deque()
        for (ki0, ki1) in pairs:
            pend.append(emit_scores(hp, ki0, ki1))
            if len(pend) >= 2:
                emit_av(av, hp, pend.popleft())
                pop_fillers()
        while pend:
            emit_av(av, hp, pend.popleft())
            pop_fillers()
        emit_norm(hp, av)


def make_outproj_fillers(tc, g, y, w):
    """Deferred output projection of window w, split into dj-half fillers."""
    nc = tc.nc
    fillers = []
    for tc4 in range(4):
        tg = 4 * w + tc4
        box = {}

        def dj_half(tg=tg, dj=0, box=box):
            if dj == 0:
                box["ysb"] = g.ysb_pool.tile([128, D], F32, tag="ysb", name="ysb")
            ysb = box["ysb"]
            py = g.pp.tile([128, 512], F32, tag="pj", bufs=2, name="py")
            for cc in range(2):
                nc.tensor.matmul(
                    out=py[:, :],
                    lhsT=g.aT_sb[cc][:, 128 * tg:128 * tg + 128],
                    rhs=g.wo_sb[cc][:, 512 * dj:512 * dj + 512],
                    start=(cc == 0), stop=(cc == 1))
            nc.vector.tensor_copy(ysb[:, 512 * dj:512 * dj + 512], py[:, :])
            if dj == 1:
                nc.sync.dma_start(out=y[128 * tg:128 * tg + 128, :], in_=ysb[:, :])

        fillers.append(lambda f=dj_half: f(dj=0))
        fillers.append(lambda f=dj_half: f(dj=1))
    return fillers


def attn_kernel(ctx, tc, y, xT, wqT, wkT, wvT, woT, n_reps=1):
    g = Ctx()
    emit_consts(ctx, tc, g)
    for rep in range(n_reps):
        for w in range(TQ):
            first = rep == 0 and w == 0
            if first:
                emit_proj(tc, g, xT, w, wqT, wkT, wvT, woT)
            else:
                emit_proj(tc, g, xT, w)
            emit_attn(tc, g, w)
            g.fillers.extend(make_outproj_fillers(tc, g, y, w))
    while g.fillers:
        g.fillers.popleft()()


_PROGRAMS = {}


def get_program(n_reps=1):
    key = (n_reps, ET_BUFS, USE_POOL_BCAST)
    if key not in _PROGRAMS:
        nc = bacc.Bacc("TRN2", target_bir_lowering=False, debug=False,
                       num_devices=N_CORES)
        xT = nc.dram_tensor("xT", [D, T], BF16, kind="ExternalInput").ap()
        wqT = nc.dram_tensor("wqT", [D, CL], BF16, kind="ExternalInput").ap()
        wkT = nc.dram_tensor("wkT", [D, CL], BF16, kind="ExternalInput").ap()
        wvT = nc.dram_tensor("wvT", [D, CL], BF16, kind="ExternalInput").ap()
        woT = nc.dram_tensor("woT", [CL, D], BF16, kind="ExternalInput").ap()
        y = nc.dram_tensor("y", [T, D], F32, kind="ExternalOutput").ap()
        with tile.TileContext(nc) as tc:
            with ExitStack() as ctx:
                attn_kernel(ctx, tc, y, xT, wqT, wkT, wvT, woT, n_reps=n_reps)
        nc.compile()
        _PROGRAMS[key] = nc
    return _PROGRAMS[key]


def get_trivial_program():
    """Minimal NEFF with the same I/O signature, for dispatch-overhead
    baseline measurement."""
    if "trivial" not in _PROGRAMS:
        nc = bacc.Bacc("TRN2", target_bir_lowering=False, debug=False,
                       num_devices=N_CORES)
        xT = nc.dram_tensor("xT", [D, T], BF16, kind="ExternalInput").ap()
        nc.dram_tensor("wqT", [D, CL], BF16, kind="ExternalInput")
        nc.dram_tensor("wkT", [D, CL], BF16, kind="ExternalInput")
        nc.dram_tensor("wvT", [D, CL], BF16, kind="ExternalInput")
        nc.dram_tensor("woT", [CL, D], BF16, kind="ExternalInput")
        y = nc.dram_tensor("y", [T, D], F32, kind="ExternalOutput").ap()
        with tile.TileContext(nc) as tc:
            with ExitStack() as ctx:
                pool = ctx.enter_context(tc.tile_pool(name="t", bufs=1))
                t = pool.tile([128, 512], BF16, tag="t", name="t")
                o = pool.tile([128, 512], F32, tag="o", name="o")
                nc.sync.dma_start(out=t[:, :], in_=xT[0:128, 0:512])
                nc.vector.tensor_copy(o[:, :], t[:, :])
                nc.sync.dma_start(out=y[0:128, 0:512], in_=o[:, :])
        nc.compile()
        _PROGRAMS["trivial"] = nc
    return _PROGRAMS["trivial"]


def make_in_maps(x, wq, wk, wv, wo):
    bf = ml_dtypes.bfloat16
    x = np.asarray(x, np.float32)
    wq, wk, wv, wo = (np.asarray(a, np.float32) for a in (wq, wk, wv, wo))
    scale = np.float32(DH ** -0.5)
    in_maps = []
    for c in range(N_CORES):
        b, gi = divmod(c, HG)
        rows = slice(gi * CL, (gi + 1) * CL)
        in_maps.append({
            "xT": np.ascontiguousarray(x[b].T).astype(bf),
            # score scale 1/sqrt(DH) folded into wq on the host
            "wqT": (np.ascontiguousarray(wq[rows].T) * scale).astype(bf),
            "wkT": np.ascontiguousarray(wk[rows].T).astype(bf),
            "wvT": np.ascontiguousarray(wv[rows].T).astype(bf),
            "woT": np.ascontiguousarray(wo[:, rows].T).astype(bf),
        })
    return in_maps


def gather(results):
    y = np.zeros((B, T, D), np.float32)
    for c in range(N_CORES):
        y[c // HG] += results[c]["y"]
    return y


def kernel(x, wq, wk, wv, wo):
    nc = get_program()
    in_maps = make_in_maps(x, wq, wk, wv, wo)
    res = run_bass_kernel_spmd(nc, in_maps, list(range(N_CORES)))
    return gather(res.results)
